# revision 1
# baseline (speedup 1.0000x reference)
"""Single-head attention (B=8, T=2048, C=512, d_k=64) on 8 Trainium2 cores.

Strategy: data-parallel over batch B — one batch element per NeuronCore,
no collectives. Per core:
  1. x tiles [128,512] DMA'd in natural layout, PE-transposed (identity
     matmul) into x^T [c,t] in SBUF (projections contract over c, which
     must sit on partitions).
  2. Q^T,K^T [64,2048] via W-as-weights matmuls; V [t,64] natural via
     x^T-as-weights; a ones-column is appended to V so the attention
     denominator falls out of the AV matmul for free.
  3. Per key-tile j: S^T = (K^T_j)^T Q^T -> PSUM [128,2048]; one ACT exp
     (scale=1/sqrt(64), no max-subtraction needed: scores ~ N(0,1));
     AV: out^T += V'_j^T @ P^T_j accumulated in PSUM over all j.
  4. Epilogue: PE-transpose out^T tiles back to [t,65], divide by the
     denominator column, DMA out.
"""

import numpy as np
from contextlib import ExitStack

import concourse.bass as bass
import concourse.tile as tile
from concourse import bacc
from concourse import mybir
from concourse.bass_utils import run_bass_kernel_spmd
from concourse.masks import make_identity

B, T, C, DK = 8, 2048, 512, 64
N_CORES = 8
FP32 = mybir.dt.float32
P = 128
TT = T // P      # 16 token tiles
CCH = C // P     # 4 contraction chunks
NB = 512         # matmul moving-operand max (fp32)
IC = T // NB     # 4 i-chunks
SCALE = 1.0 / np.sqrt(np.float32(DK))

_cached = {}


def _build_nc():
    nc = bacc.Bacc("TRN2", target_bir_lowering=False, debug=False)
    x_d = nc.declare_dram_parameter("x", [T, C], FP32, isOutput=False)
    wq_d = nc.declare_dram_parameter("Wq", [C, DK], FP32, isOutput=False)
    wk_d = nc.declare_dram_parameter("Wk", [C, DK], FP32, isOutput=False)
    wv_d = nc.declare_dram_parameter("Wv", [C, DK], FP32, isOutput=False)
    out_d = nc.declare_dram_parameter("out", [T, DK], FP32, isOutput=True)

    x_t = x_d.rearrange("(tt p) c -> tt p c", p=P)          # [16,128,512]
    out_t = out_d.rearrange("(tt p) d -> tt p d", p=P)      # [16,128,64]

    with ExitStack() as ctx:
        tc = ctx.enter_context(tile.TileContext(nc))
        const = ctx.enter_context(tc.tile_pool(name="const", bufs=1))

        identity = const.tile([P, P], FP32)
        make_identity(nc, identity)

        # --- weights to SBUF, chunked over c ---
        wq_s = const.tile([P, CCH, DK], FP32)
        wk_s = const.tile([P, CCH, DK], FP32)
        wv_s = const.tile([P, CCH, DK], FP32)
        nc.sync.dma_start(out=wq_s, in_=wq_d.rearrange("(ch p) d -> p ch d", p=P))
        nc.sync.dma_start(out=wk_s, in_=wk_d.rearrange("(ch p) d -> p ch d", p=P))
        nc.sync.dma_start(out=wv_s, in_=wv_d.rearrange("(ch p) d -> p ch d", p=P))

        xT = const.tile([P, CCH, T], FP32)          # x^T, 32KB/part
        v_s = const.tile([P, TT, DK + 1], FP32)     # V with ones col
        nc.vector.memset(v_s, 1.0)
        qT = const.tile([DK, T], FP32)
        kT = const.tile([DK, T], FP32)

        # --- phase 1: load x, transpose into xT; V per t-tile ---
        with (
            tc.tile_pool(name="xload", bufs=3) as xload,
            tc.tile_pool(name="tpsum", bufs=2, space="PSUM") as tpsum,
            tc.tile_pool(name="vpsum", bufs=2, space="PSUM") as vpsum,
        ):
            for tt in range(TT):
                x_tile = xload.tile([P, C], FP32, tag="x_tile")
                nc.sync.dma_start(out=x_tile, in_=x_t[tt])
                for ch in range(CCH):
                    ps = tpsum.tile([P, P], FP32, tag="tps")
                    nc.tensor.transpose(ps, x_tile[:, ch * P:(ch + 1) * P], identity)
                    nc.vector.tensor_copy(out=xT[:, ch, tt * P:(tt + 1) * P], in_=ps)
                pv = vpsum.tile([P, DK], FP32, tag="pv")
                for ch in range(CCH):
                    nc.tensor.matmul(
                        pv, lhsT=xT[:, ch, tt * P:(tt + 1) * P], rhs=wv_s[:, ch, :],
                        start=(ch == 0), stop=(ch == CCH - 1))
                nc.vector.tensor_copy(out=v_s[:, tt, 0:DK], in_=pv)

            # --- phase 2: Q^T, K^T projections ---
            for ic in range(IC):
                pq = vpsum.tile([DK, NB], FP32, tag="pq")
                pk = vpsum.tile([DK, NB], FP32, tag="pk")
                for ch in range(CCH):
                    nc.tensor.matmul(
                        pq, lhsT=wq_s[:, ch, :], rhs=xT[:, ch, ic * NB:(ic + 1) * NB],
                        start=(ch == 0), stop=(ch == CCH - 1))
                for ch in range(CCH):
                    nc.tensor.matmul(
                        pk, lhsT=wk_s[:, ch, :], rhs=xT[:, ch, ic * NB:(ic + 1) * NB],
                        start=(ch == 0), stop=(ch == CCH - 1))
                nc.vector.tensor_copy(out=qT[:, ic * NB:(ic + 1) * NB], in_=pq)
                nc.vector.tensor_copy(out=kT[:, ic * NB:(ic + 1) * NB], in_=pk)

        # --- main loop: S^T -> exp -> AV accumulate ---
        with (
            tc.tile_pool(name="spsum", bufs=1, space="PSUM") as spsum,
            tc.tile_pool(name="opsum", bufs=1, space="PSUM") as opsum,
            tc.tile_pool(name="ppool", bufs=2) as ppool,
        ):
            o_ps = []
            for ic in range(IC):
                o_tile = opsum.tile([DK + 1, NB], FP32, tag=f"ops{ic}")
                o_ps.append(o_tile)
            for j in range(TT):
                pT = ppool.tile([P, T], FP32, tag="pT")
                for h in range(2):
                    s_ps = spsum.tile([P, T // 2], FP32, tag="sps", bufs=2)
                    for ic in range(2):
                        icg = h * 2 + ic
                        nc.tensor.matmul(
                            s_ps[:, ic * NB:(ic + 1) * NB],
                            lhsT=kT[:, j * P:(j + 1) * P],
                            rhs=qT[:, icg * NB:(icg + 1) * NB],
                            start=True, stop=True)
                    nc.scalar.activation(
                        out=pT[:, h * (T // 2):(h + 1) * (T // 2)], in_=s_ps,
                        func=mybir.ActivationFunctionType.Exp, scale=float(SCALE))
                for ic in range(IC):
                    nc.tensor.matmul(
                        o_ps[ic], lhsT=v_s[:, j, :], rhs=pT[:, ic * NB:(ic + 1) * NB],
                        start=(j == 0), stop=(j == TT - 1), skip_group_check=True)

            # --- epilogue: transpose out^T back, normalize, store ---
            oT_s = ppool.tile([DK + 1, T], FP32, tag="oTs", bufs=1)
            for ic in range(IC):
                nc.vector.tensor_copy(out=oT_s[:, ic * NB:(ic + 1) * NB], in_=o_ps[ic])

        with (
            tc.tile_pool(name="epsum", bufs=2, space="PSUM") as epsum,
            tc.tile_pool(name="outp", bufs=3) as outp,
        ):
            for tt in range(TT):
                ot_ps = epsum.tile([P, DK + 1], FP32, tag="otps")
                nc.tensor.transpose(
                    ot_ps, oT_s[:, tt * P:(tt + 1) * P], identity[0:DK + 1, 0:DK + 1])
                recip = outp.tile([P, 1], FP32, tag="recip")
                nc.vector.reciprocal(recip, ot_ps[:, DK:DK + 1])
                o_tile2 = outp.tile([P, DK], FP32, tag="otile")
                nc.vector.tensor_scalar_mul(o_tile2, ot_ps[:, 0:DK], recip)
                nc.sync.dma_start(out=out_t[tt], in_=o_tile2)

    nc.compile()
    return nc


def _get_nc():
    if "nc" not in _cached:
        _cached["nc"] = _build_nc()
    return _cached["nc"]


def kernel(x, Wq, Wk, Wv, **run_kwargs):
    x = np.asarray(x, dtype=np.float32)
    Wq = np.asarray(Wq, dtype=np.float32)
    Wk = np.asarray(Wk, dtype=np.float32)
    Wv = np.asarray(Wv, dtype=np.float32)
    nc = _get_nc()
    in_maps = [
        {"x": np.ascontiguousarray(x[b]), "Wq": Wq, "Wk": Wk, "Wv": Wv}
        for b in range(B)
    ]
    res = run_bass_kernel_spmd(nc, in_maps, list(range(N_CORES)), **run_kwargs)
    out = np.stack([res.results[b]["out"] for b in range(B)], axis=0)
    if run_kwargs:
        _cached["last_result"] = res
    return out



# revision 4
# speedup vs baseline: 1.4246x; 1.4246x over previous
"""Single-head attention (B=8, T=2048, C=512, d_k=64) on 8 Trainium2 cores.

Strategy: data-parallel over batch B — one batch element per NeuronCore,
no collectives. Matmuls run in float32r (replicated-fp32: 1 cycle/row at
moving-dim >= 256 vs 4 for plain fp32). The BIR verifier requires every
producer feeding an fp32r matmul to round its output to fp32r, so those
SBUF tiles are declared float32r and written by DVE/ACT ops; PE
transposes stay plain fp32. Per core:
  1. x tiles [128,512] DMA'd in natural layout, PE-transposed (identity
     matmul) into x^T [c,t] in SBUF (projections contract over c, which
     must sit on partitions).
  2. Q^T,K^T,V^T [64,2048] via W-as-weights matmuls over x^T (moving
     N=512); V^T is PE-transposed back to V [t,64] tiles, with a ones
     column appended so the attention denominator falls out of the AV
     matmul for free.
  3. Per key-tile j: S^T = (K^T_j)^T Q^T -> PSUM [128,2048]; one ACT exp
     (scale=1/sqrt(64), no max-subtraction needed: scores ~ N(0,1));
     AV: out^T += V'_j^T @ P^T_j accumulated in PSUM over all j.
  4. Epilogue: PE-transpose out^T tiles back to [t,65], divide by the
     denominator column, DMA out.
"""

import numpy as np
from contextlib import ExitStack

import concourse.bass as bass
import concourse.tile as tile
from concourse import bacc
from concourse import mybir
from concourse.bass_utils import run_bass_kernel_spmd
from concourse.masks import make_identity

B, T, C, DK = 8, 2048, 512, 64
N_CORES = 8
FP32 = mybir.dt.float32
FP32R = mybir.dt.float32r
P = 128
TT = T // P      # 16 token tiles
CCH = C // P     # 4 contraction chunks
NB = 512         # matmul moving-operand max (4-byte dtypes)
IC = T // NB     # 4 i-chunks
SCALE = 1.0 / np.sqrt(np.float32(DK))

_cached = {}


def _build_nc():
    nc = bacc.Bacc("TRN2", target_bir_lowering=False, debug=False)
    x_d = nc.declare_dram_parameter("x", [T, C], FP32, isOutput=False)
    wq_d = nc.declare_dram_parameter("Wq", [C, DK], FP32, isOutput=False)
    wk_d = nc.declare_dram_parameter("Wk", [C, DK], FP32, isOutput=False)
    wv_d = nc.declare_dram_parameter("Wv", [C, DK], FP32, isOutput=False)
    out_d = nc.declare_dram_parameter("out", [T, DK], FP32, isOutput=True)

    x_t = x_d.rearrange("(tt p) c -> tt p c", p=P)          # [16,128,512]
    out_t = out_d.rearrange("(tt p) d -> tt p d", p=P)      # [16,128,64]

    with ExitStack() as ctx:
        tc = ctx.enter_context(tile.TileContext(nc))
        const = ctx.enter_context(tc.tile_pool(name="const", bufs=1))

        identity = const.tile([P, P], FP32)
        make_identity(nc, identity)

        # --- weights to SBUF (fp32 staging), rounded to fp32r via DVE ---
        wq_s = const.tile([P, CCH, DK], FP32R)
        wk_s = const.tile([P, CCH, DK], FP32R)
        wv_s = const.tile([P, CCH, DK], FP32R)
        with tc.tile_pool(name="wstage", bufs=1) as wstage:
            for (w_d, w_s) in ((wq_d, wq_s), (wk_d, wk_s), (wv_d, wv_s)):
                w_stg = wstage.tile([P, CCH, DK], FP32, tag=f"stg{w_d.name}")
                nc.sync.dma_start(out=w_stg, in_=w_d.rearrange("(ch p) d -> p ch d", p=P))
                nc.vector.tensor_copy(out=w_s, in_=w_stg)

        xT = const.tile([P, CCH, T], FP32R)         # x^T, 32KB/part
        v_s = const.tile([P, TT, DK + 1], FP32R)    # V with ones col
        ones_col = const.tile([P, TT], FP32)
        nc.vector.memset(ones_col, 1.0)
        nc.vector.tensor_copy(
            out=v_s[:, :, DK:DK + 1].rearrange("p tt one -> p (tt one)"),
            in_=ones_col)
        qT = const.tile([DK, T], FP32R)
        kT = const.tile([DK, T], FP32R)
        vT = const.tile([DK, T], FP32)

        # --- phase 1: load x, transpose into xT (plain fp32 transposes) ---
        with (
            tc.tile_pool(name="xload", bufs=3) as xload,
            tc.tile_pool(name="tpsum", bufs=4, space="PSUM") as tpsum,
        ):
            for tt in range(TT):
                x_tile = xload.tile([P, C], FP32, tag="x_tile")
                nc.sync.dma_start(out=x_tile, in_=x_t[tt])
                for ch in range(CCH):
                    ps = tpsum.tile([P, P], FP32, tag="tps")
                    nc.tensor.transpose(ps, x_tile[:, ch * P:(ch + 1) * P], identity)
                    nc.vector.tensor_copy(out=xT[:, ch, tt * P:(tt + 1) * P], in_=ps)

        # --- phase 2: Q^T, K^T, V^T projections (contract over c) ---
        with tc.tile_pool(name="ppsum", bufs=3, space="PSUM") as ppsum:
            for (w_s, dst) in ((wq_s, qT), (wk_s, kT), (wv_s, vT)):
                for ic in range(IC):
                    pp = ppsum.tile([DK, NB], FP32, tag="pp")
                    for ch in range(CCH):
                        nc.tensor.matmul(
                            pp, lhsT=w_s[:, ch, :],
                            rhs=xT[:, ch, ic * NB:(ic + 1) * NB],
                            start=(ch == 0), stop=(ch == CCH - 1))
                    nc.vector.tensor_copy(out=dst[:, ic * NB:(ic + 1) * NB], in_=pp)

            # V^T -> V tiles [128, 64] (transpose back; col 64 stays ones)
            for tt in range(TT):
                pv = ppsum.tile([P, DK], FP32, tag="pv")
                nc.tensor.transpose(
                    pv, vT[:, tt * P:(tt + 1) * P], identity[0:DK, 0:DK])
                nc.vector.tensor_copy(out=v_s[:, tt, 0:DK], in_=pv)

        # --- main loop: S^T -> exp -> AV accumulate ---
        with (
            tc.tile_pool(name="spsum", bufs=1, space="PSUM") as spsum,
            tc.tile_pool(name="opsum", bufs=1, space="PSUM") as opsum,
            tc.tile_pool(name="ppool", bufs=2) as ppool,
        ):
            o_ps = []
            for ic in range(IC):
                o_tile = opsum.tile([DK + 1, NB], FP32, tag=f"ops{ic}")
                o_ps.append(o_tile)
            for j in range(TT):
                pT = ppool.tile([P, T], FP32R, tag="pT")
                for h in range(2):
                    s_ps = spsum.tile([P, T // 2], FP32, tag="sps", bufs=2)
                    for ic in range(2):
                        icg = h * 2 + ic
                        nc.tensor.matmul(
                            s_ps[:, ic * NB:(ic + 1) * NB],
                            lhsT=kT[:, j * P:(j + 1) * P],
                            rhs=qT[:, icg * NB:(icg + 1) * NB],
                            start=True, stop=True)
                    nc.scalar.activation(
                        out=pT[:, h * (T // 2):(h + 1) * (T // 2)], in_=s_ps,
                        func=mybir.ActivationFunctionType.Exp, scale=float(SCALE))
                for ic in range(IC):
                    nc.tensor.matmul(
                        o_ps[ic], lhsT=v_s[:, j, :],
                        rhs=pT[:, ic * NB:(ic + 1) * NB],
                        start=(j == 0), stop=(j == TT - 1), skip_group_check=True)

            # --- epilogue: transpose out^T back, normalize, store ---
            oT_s = ppool.tile([DK + 1, T], FP32, tag="oTs", bufs=1)
            for ic in range(IC):
                nc.vector.tensor_copy(out=oT_s[:, ic * NB:(ic + 1) * NB], in_=o_ps[ic])

        with (
            tc.tile_pool(name="epsum", bufs=2, space="PSUM") as epsum,
            tc.tile_pool(name="outp", bufs=3) as outp,
        ):
            for tt in range(TT):
                ot_ps = epsum.tile([P, DK + 1], FP32, tag="otps")
                nc.tensor.transpose(
                    ot_ps, oT_s[:, tt * P:(tt + 1) * P], identity[0:DK + 1, 0:DK + 1])
                recip = outp.tile([P, 1], FP32, tag="recip")
                nc.vector.reciprocal(recip, ot_ps[:, DK:DK + 1])
                o_tile2 = outp.tile([P, DK], FP32, tag="otile")
                nc.vector.tensor_scalar_mul(o_tile2, ot_ps[:, 0:DK], recip)
                nc.sync.dma_start(out=out_t[tt], in_=o_tile2)

    nc.compile()
    return nc


def _get_nc():
    if "nc" not in _cached:
        _cached["nc"] = _build_nc()
    return _cached["nc"]


def kernel(x, Wq, Wk, Wv, **run_kwargs):
    x = np.asarray(x, dtype=np.float32)
    Wq = np.asarray(Wq, dtype=np.float32)
    Wk = np.asarray(Wk, dtype=np.float32)
    Wv = np.asarray(Wv, dtype=np.float32)
    nc = _get_nc()
    in_maps = [
        {"x": np.ascontiguousarray(x[b]), "Wq": Wq, "Wk": Wk, "Wv": Wv}
        for b in range(B)
    ]
    res = run_bass_kernel_spmd(nc, in_maps, list(range(N_CORES)), **run_kwargs)
    out = np.stack([res.results[b]["out"] for b in range(B)], axis=0)
    if run_kwargs:
        _cached["last_result"] = res
    return out
